# revision 13
# baseline (speedup 1.0000x reference)
"""Trainium2 Bass kernel for nn_Block_71820443124300 (moe_routing).

Reference computation (B=2, S=2048, D=512, H=8, E=8, F=4D=2048):
  attention (no mask) -> +x -> global layernorm -> dense MoE over all
  experts with softmax(router-mean) gates -> +x -> global layernorm.

Sharding over 8 cores: two groups of 4 cores, one per batch element.
Within a group of 4 cores:
  - attention is head-sharded (2 of 8 heads per core), partial attention
    output all-reduced (with x/4 + b2/4 folded in => z = attn + x).
  - global LN stats via tiny 8-rank all-reduce.
  - MoE is expert-sharded (2 of 8 experts per core); gate-weighted
    partials (+ x_ln1/4 residual via scaled-identity matmul) are
    reduce-scattered so core j ends up owning token block j.
  - final LN stats via tiny 8-rank all-reduce; each core outputs its
    [512 token x 512 feature] block (transposed); host reassembles.

All matmuls run in bf16 (fp32 PSUM accumulation); layernorm statistics,
softmax denominators and residual paths are fp32 except where noted.
Everything is computed in feature-on-partition ("transposed") layout so
no on-device transposes are needed anywhere.
"""

import numpy as np
import ml_dtypes

import concourse.bass as bass
import concourse.bacc as bacc
import concourse.mybir as mybir
import concourse.tile as tile
from concourse.bass_utils import run_bass_kernel_spmd

F32 = mybir.dt.float32
BF16 = mybir.dt.bfloat16
AF = mybir.ActivationFunctionType
ALU = mybir.AluOpType
AX = mybir.AxisListType

B, S, D, H, E = 2, 2048, 512, 8, 8
F = 4 * D            # 2048
DH = D // H          # 64
EPS = 1e-12
N_CORES = 8
DT = D // 128        # 4 D-tiles
ST = S // 128        # 16 token tiles
FT = F // 128        # 16 f tiles
NTOT = float(B * S * D)   # layernorm element count

GROUPS4 = [[0, 1, 2, 3], [4, 5, 6, 7]]
GROUP8 = [list(range(8))]


def bf(x):
    return np.asarray(x, dtype=ml_dtypes.bfloat16)


def f32(x):
    return np.ascontiguousarray(np.asarray(x, dtype=np.float32))


# ----------------------------------------------------------------------------
# device program
# ----------------------------------------------------------------------------

def _declare_io(nc):
    t = {}
    def inp(name, shape, dt):
        t[name] = nc.dram_tensor(name, list(shape), dt, kind="ExternalInput")
    inp("xT_bf", [DT, 128, S], BF16)       # x[b].T as D-tiles
    inp("xq", [DT, 128, S], F32)           # 0.25*x[b].T + 0.25*b2 (per D row)
    inp("w1h", [DT, 128, 384], BF16)       # [q2h | k2h | v2h] columns of W1
    inp("b1qk", [128, 2], F32)             # q,k bias per dh-partition
    inp("bv_row", [1, 128], BF16)          # v bias row
    inp("w2h", [128, 512], BF16)           # W2 rows for this core's 2 heads
    inp("wr_r", [128, 32], F32)            # Wr tiles: col block t = Wr[128t:,:]
    inp("br_r", [1, 8], F32)
    inp("lnw1", [128, DT], F32)
    inp("lnb1", [128, DT], F32)
    inp("lnw2", [128, DT], F32)
    inp("lnb2", [128, DT], F32)
    inp("wfc_in", [2, DT, 128, F], BF16)   # per local expert, D-tiled
    inp("bfc_r", [2, 128, FT], F32)        # per-f bias, f-tiled
    inp("wproj_in", [2, 128, FT * 512], BF16)  # f-tile-major [128, 8192]
    inp("bproj_r", [2, 128, DT], F32)
    inp("esel", [2, 1, 8], F32)            # one-hot rows selecting local experts
    t["outT"] = nc.dram_tensor("outT", [D, 512], F32, kind="ExternalOutput")
    return t


def _emit_body(nc, tc, t, sb, ph, ps, dram):
    """Emit one full forward pass. sb: SBUF pool, ph: psum pool with
    [128,1024] slots (bufs=3), ps: psum pool with [128,512] slots (bufs=2),
    dram: DRAM pool for collective bounce buffers."""

    # ---------------- constants & small tiles ----------------
    ones_col_f = sb.tile([128, 1], F32)
    nc.vector.memset(ones_col_f[:], 1.0)
    ones_row_f = sb.tile([1, 128], F32)
    nc.vector.memset(ones_row_f[:], 1.0)
    ones_row_b = sb.tile([1, 128], BF16)
    nc.vector.memset(ones_row_b[:], 1.0)
    id025 = sb.tile([128, 128], BF16)
    nc.gpsimd.memset(id025[:], 0.0)
    nc.gpsimd.affine_select(
        out=id025[:], in_=id025[:], compare_op=ALU.not_equal, fill=0.25,
        base=0, pattern=[[-1, 128]], channel_multiplier=1,
    )

    b1qk = sb.tile([128, 2], F32)
    nc.sync.dma_start(b1qk[:], t["b1qk"].ap())
    bv_row = sb.tile([1, 128], BF16)
    nc.sync.dma_start(bv_row[:], t["bv_row"].ap())
    w2h = sb.tile([128, 512], BF16)
    nc.sync.dma_start(w2h[:], t["w2h"].ap())
    wr_r = sb.tile([128, 32], F32)
    nc.sync.dma_start(wr_r[:], t["wr_r"].ap())
    br_r = sb.tile([1, 8], F32)
    nc.sync.dma_start(br_r[:], t["br_r"].ap())
    lnv = {}
    for nm in ("lnw1", "lnb1", "lnw2", "lnb2"):
        lnv[nm] = sb.tile([128, DT], F32, name=nm + "_sb")
        nc.sync.dma_start(lnv[nm][:], t[nm].ap())
    esel = sb.tile([1, 2, 8], F32)
    for l in range(2):
        nc.sync.dma_start(esel[:, l, :], t["esel"].ap()[l])

    # ---------------- big weight/activation loads ----------------
    xT = []
    for dt_ in range(DT):
        x_t = sb.tile([128, S], BF16, name=f"xT{dt_}")
        nc.sync.dma_start(x_t[:], t["xT_bf"].ap()[dt_])
        xT.append(x_t)
    xq = []
    for dt_ in range(DT):
        xq_t = sb.tile([128, S], F32, name=f"xq{dt_}", tag=f"zx{dt_}")
        nc.sync.dma_start(xq_t[:], t["xq"].ap()[dt_])
        xq.append(xq_t)
    w1h = []
    for dt_ in range(DT):
        w_t = sb.tile([128, 384], BF16, name=f"w1h{dt_}")
        nc.sync.dma_start(w_t[:], t["w1h"].ap()[dt_])
        w1h.append(w_t)
    wfc = [[None] * DT for _ in range(2)]
    for l in range(2):
        for dt_ in range(DT):
            w_t = sb.tile([128, F], BF16, name=f"wfc{l}_{dt_}")
            nc.sync.dma_start(w_t[:], t["wfc_in"].ap()[l, dt_])
            wfc[l][dt_] = w_t
    wproj = []
    for l in range(2):
        w_t = sb.tile([128, FT * 512], BF16, name=f"wproj{l}")
        nc.sync.dma_start(w_t[:], t["wproj_in"].ap()[l])
        wproj.append(w_t)
    bfc = sb.tile([128, 2, FT], F32)
    bproj = sb.tile([128, 2, DT], F32)
    for l in range(2):
        nc.sync.dma_start(bfc[:, l, :], t["bfc_r"].ap()[l])
        nc.sync.dma_start(bproj[:, l, :], t["bproj_r"].ap()[l])

    # ---------------- qkv ----------------
    # qT/kT: [128 (2 heads x 64 dh), S] = W1 slice^T @ x^T
    qT = sb.tile([128, S], BF16)
    kT = sb.tile([128, S], BF16)
    for (dst, col0, bcol) in ((qT, 0, 0), (kT, 128, 1)):
        for half in range(2):
            pqk = ph.tile([128, 1024], F32, name=f"pqk{col0}_{half}", tag="ph")
            for kt in range(DT):
                for qc in range(2):
                    nc.tensor.matmul(
                        pqk[:, qc * 512:(qc + 1) * 512],
                        w1h[kt][:, col0:col0 + 128],
                        xT[kt][:, half * 1024 + qc * 512:half * 1024 + (qc + 1) * 512],
                        start=(kt == 0), stop=(kt == DT - 1),
                    )
            nc.scalar.activation(
                dst[:, half * 1024:(half + 1) * 1024], pqk[:],
                AF.Identity, bias=b1qk[:, bcol:bcol + 1],
            )

    # v (un-transposed): [tok, 128 dv] per token tile, then packed into
    # v_aug [128, ST*130]: per tile [h0 64v | 1 | h1 64v | 1]
    v_aug = sb.tile([128, ST * 130], BF16)
    ones_sl = v_aug[:].rearrange("p (n c) -> p n c", c=65)[:, :, 64:65]
    nc.vector.memset(ones_sl, 1.0)
    for st_ in range(ST):
        pv = ps.tile([128, 128], F32, name=f"pv{st_}", tag="ps")
        for kt in range(DT):
            nc.tensor.matmul(
                pv[:], xT[kt][:, st_ * 128:(st_ + 1) * 128],
                w1h[kt][:, 256:384],
                start=(kt == 0), stop=False,
            )
        nc.tensor.matmul(pv[:], ones_row_b[:], bv_row[:], start=False, stop=True)
        dst = v_aug[:, st_ * 130:st_ * 130 + 130].rearrange(
            "p (h c) -> p h c", c=65)[:, :, 0:64]
        nc.scalar.activation(dst, pv[:].rearrange("p (h c) -> p h c", c=64),
                             AF.Identity)

    # ---------------- attention: scores -> softmax -> AV ----------------
    oT = sb.tile([128, S], BF16)   # [2*64 feat, q] normalized attention out
    for h in range(2):
        hp = h * 64
        for qh in range(2):
            q0 = qh * 1024
            po = ph.tile([128, 1024], F32, name=f"po{h}_{qh}", tag="ph")
            for kt in range(ST):
                psc = ph.tile([128, 1024], F32, name=f"psc{h}_{qh}_{kt}", tag="ph")
                for qc in range(2):
                    nc.tensor.matmul(
                        psc[:, qc * 512:(qc + 1) * 512],
                        kT[hp:hp + 64, kt * 128:(kt + 1) * 128],
                        qT[hp:hp + 64, q0 + qc * 512:q0 + (qc + 1) * 512],
                        start=True, stop=True,
                    )
                wexp = sb.tile([128, 1024], BF16, name=f"wexp{h}{qh}{kt}",
                               tag="wexp", bufs=2)
                nc.scalar.activation(wexp[:], psc[:], AF.Exp, scale=0.125)
                for qc in range(2):
                    nc.tensor.matmul(
                        po[0:65, qc * 512:(qc + 1) * 512],
                        v_aug[:, kt * 130 + h * 65: kt * 130 + h * 65 + 65],
                        wexp[:, qc * 512:(qc + 1) * 512],
                        start=(kt == 0), stop=(kt == ST - 1),
                    )
            # normalize: rows 0..63 divided by row 64
            recip = sb.tile([1, 1024], F32, name=f"rc{h}{qh}", tag="recip", bufs=2)
            nc.vector.reciprocal(recip[:], po[64:65, :])
            pbc = ph.tile([128, 1024], F32, name=f"pbc{h}_{qh}", tag="ph")
            for qc in range(2):
                nc.tensor.matmul(
                    pbc[0:64, qc * 512:(qc + 1) * 512],
                    ones_row_f[:, 0:64],
                    recip[:, qc * 512:(qc + 1) * 512],
                    start=True, stop=True,
                )
            bc_sb = sb.tile([64, 1024], F32, name=f"bcsb{h}{qh}", tag="bcsb",
                            bufs=1)
            nc.scalar.activation(bc_sb[:], pbc[0:64, :], AF.Copy)
            nc.vector.tensor_tensor(
                oT[hp:hp + 64, q0:q0 + 1024], po[0:64, :], bc_sb[:], ALU.mult)

    # ---------------- attention out-proj + residual; z all-reduce --------
    ar_in = dram.tile([D, S], F32)
    ar_out = dram.tile([D, S], F32)
    for mt in range(DT):
        for half in range(2):
            pa = ph.tile([128, 1024], F32, name=f"pa{mt}_{half}", tag="ph")
            for qc in range(2):
                nc.tensor.matmul(
                    pa[:, qc * 512:(qc + 1) * 512],
                    w2h[:, mt * 128:(mt + 1) * 128],
                    oT[:, half * 1024 + qc * 512:half * 1024 + (qc + 1) * 512],
                    start=True, stop=True,
                )
            # += 0.25*x^T + 0.25*b2 (so the 4-rank all-reduce yields z = a+x)
            arsb = sb.tile([128, 1024], F32, name=f"arsb{mt}_{half}",
                           tag="arsb", bufs=3)
            nc.vector.tensor_tensor(
                arsb[:], pa[:], xq[mt][:, half * 1024:(half + 1) * 1024], ALU.add)
            nc.sync.dma_start(
                ar_in[mt * 128:(mt + 1) * 128, half * 1024:(half + 1) * 1024],
                arsb[:])
    nc.gpsimd.collective_compute(
        "AllReduce", ALU.add, replica_groups=GROUPS4,
        ins=[ar_in.opt()], outs=[ar_out.opt()],
    )
    zT = []
    for dt_ in range(DT):
        z_t = sb.tile([128, S], F32, name=f"zT{dt_}", tag=f"zx{dt_}")
        nc.sync.dma_start(z_t[:], ar_out[dt_ * 128:(dt_ + 1) * 128, :])
        zT.append(z_t)

    # ---------------- layernorm-1 statistics (global over both b) --------
    # cols 0..7: per-(dt,half) row sums; cols 8..15: row sums of squares
    stats_in = sb.tile([128, 16], F32)
    for dt_ in range(DT):
        for hf in range(2):
            zsl = zT[dt_][:, hf * 1024:(hf + 1) * 1024]
            scr = sb.tile([128, 1024], F32, name=f"sqs{dt_}{hf}", tag="arsb",
                          bufs=3)
            c = 2 * dt_ + hf
            nc.scalar.activation(scr[:], zsl, AF.Square,
                                 accum_out=stats_in[:, 8 + c:9 + c])
            nc.vector.reduce_sum(stats_in[:, c:c + 1], zsl, axis=AX.X)
    pst = ps.tile([1, 16], F32, name="pst", tag="ps")
    nc.tensor.matmul(pst[:], ones_col_f[:], stats_in[:], start=True, stop=True)
    st8 = sb.tile([1, 16], F32)
    nc.vector.tensor_copy(st8[:], pst[:])
    pay = sb.tile([1, 16], F32)
    nc.vector.memset(pay[:], 0.0)
    s1l = sb.tile([1, 1], F32)
    s2l = sb.tile([1, 1], F32)
    nc.vector.reduce_sum(s1l[:], st8[:, 0:8], axis=AX.X)
    nc.vector.reduce_sum(s2l[:], st8[:, 8:16], axis=AX.X)
    nc.vector.tensor_scalar(pay[:, 0:1], s1l[:], 0.25, None, op0=ALU.mult)
    nc.vector.tensor_scalar(pay[:, 1:2], s2l[:], 0.25, None, op0=ALU.mult)
    ar1b_in = dram.tile([1, 16], F32)
    ar1b_out = dram.tile([1, 16], F32)
    nc.sync.dma_start(ar1b_in[:], pay[:])
    nc.gpsimd.collective_compute(
        "AllReduce", ALU.add, replica_groups=GROUP8,
        ins=[ar1b_in.opt()], outs=[ar1b_out.opt()],
    )
    gst = sb.tile([1, 16], F32)
    nc.sync.dma_start(gst[:], ar1b_out[:])

    def ln_scale_shift(gstats, lnw, lnb, tag):
        """gstats [1,16] with [0]=sum,[1]=sumsq -> per-partition scale/shift
        tiles s_d, t_d [128, DT], plus (mu_b, rstd_b) [128,1] broadcasts."""
        mu = sb.tile([1, 1], F32, name=f"mu_{tag}")
        nc.vector.tensor_scalar(mu[:], gstats[:, 0:1], 1.0 / NTOT, None,
                                op0=ALU.mult)
        ex2 = sb.tile([1, 1], F32, name=f"ex2_{tag}")
        nc.vector.tensor_scalar(ex2[:], gstats[:, 1:2], 1.0 / NTOT, None,
                                op0=ALU.mult)
        musq = sb.tile([1, 1], F32, name=f"musq_{tag}")
        nc.vector.tensor_tensor(musq[:], mu[:], mu[:], ALU.mult)
        var = sb.tile([1, 1], F32, name=f"var_{tag}")
        nc.vector.tensor_tensor(var[:], ex2[:], musq[:], ALU.subtract)
        # unbiased (ddof=1): * N/(N-1), then +eps
        nc.vector.tensor_scalar(var[:], var[:], NTOT / (NTOT - 1.0), EPS,
                                op0=ALU.mult, op1=ALU.add)
        std = sb.tile([1, 1], F32, name=f"std_{tag}")
        nc.scalar.activation(std[:], var[:], AF.Sqrt)
        rstd = sb.tile([1, 1], F32, name=f"rstd_{tag}")
        nc.vector.reciprocal(rstd[:], std[:])
        two = sb.tile([1, 2], F32, name=f"two_{tag}")
        nc.vector.tensor_copy(two[:, 0:1], mu[:])
        nc.vector.tensor_copy(two[:, 1:2], rstd[:])
        pb = ps.tile([128, 2], F32, name=f"pb_{tag}", tag="ps")
        nc.tensor.matmul(pb[:], ones_row_f[:], two[:], start=True, stop=True)
        mr_b = sb.tile([128, 2], F32, name=f"mrb_{tag}")
        nc.vector.tensor_copy(mr_b[:], pb[:])
        s_d = sb.tile([128, DT], F32, name=f"sd_{tag}")
        nc.vector.tensor_scalar(s_d[:], lnw[:], mr_b[:, 1:2], None, op0=ALU.mult)
        t_d = sb.tile([128, DT], F32, name=f"td_{tag}")
        nc.vector.tensor_scalar(t_d[:], s_d[:], mr_b[:, 0:1], None, op0=ALU.mult)
        nc.vector.tensor_tensor(t_d[:], lnb[:], t_d[:], ALU.subtract)
        return s_d, t_d

    s1_d, t1_d = ln_scale_shift(gst, lnv["lnw1"], lnv["lnb1"], "ln1")

    xln = []
    for dt_ in range(DT):
        xl_t = sb.tile([128, S], BF16, name=f"xln{dt_}")
        nc.vector.tensor_scalar(
            xl_t[:], zT[dt_][:], s1_d[:, dt_:dt_ + 1], t1_d[:, dt_:dt_ + 1],
            op0=ALU.mult, op1=ALU.add)
        xln.append(xl_t)

    # ---------------- router ----------------
    zrm = sb.tile([128, DT], F32)
    halves = stats_in[:, 0:8].rearrange("p (d two) -> p d two", two=2)
    nc.vector.tensor_tensor(zrm[:], halves[:, :, 0], halves[:, :, 1], ALU.add)
    nc.vector.tensor_scalar(zrm[:], zrm[:], 1.0 / S, None, op0=ALU.mult)
    m1 = sb.tile([128, DT], F32)
    nc.vector.tensor_tensor(m1[:], zrm[:], s1_d[:], ALU.mult)
    nc.vector.tensor_tensor(m1[:], m1[:], t1_d[:], ALU.add)
    plg = ps.tile([1, 8], F32, name="plg", tag="ps")
    for dt_ in range(DT):
        nc.tensor.matmul(plg[:], m1[:, dt_:dt_ + 1], wr_r[:, dt_ * 8:(dt_ + 1) * 8],
                         start=(dt_ == 0), stop=(dt_ == DT - 1))
    lg = sb.tile([1, 8], F32)
    nc.vector.tensor_copy(lg[:], plg[:])
    nc.vector.tensor_tensor(lg[:], lg[:], br_r[:], ALU.add)
    gex = sb.tile([1, 8], F32)
    gden = sb.tile([1, 1], F32)
    nc.scalar.activation(gex[:], lg[:], AF.Exp, accum_out=gden[:])
    grec = sb.tile([1, 1], F32)
    nc.vector.reciprocal(grec[:], gden[:])
    gates = sb.tile([1, 8], F32)
    nc.vector.tensor_scalar(gates[:], gex[:], grec[:], None, op0=ALU.mult)
    g_b = []
    for l in range(2):
        gtmp = sb.tile([1, 8], F32, name=f"gtmp{l}")
        nc.vector.tensor_tensor(gtmp[:], gates[:], esel[:, l, :], ALU.mult)
        gl = sb.tile([1, 1], F32, name=f"gl{l}")
        nc.vector.reduce_sum(gl[:], gtmp[:], axis=AX.X)
        pgb = ps.tile([128, 1], F32, name=f"pgb{l}", tag="ps")
        nc.tensor.matmul(pgb[:], ones_row_f[:], gl[:], start=True, stop=True)
        gb = sb.tile([128, 1], F32, name=f"gb{l}")
        nc.vector.tensor_copy(gb[:], pgb[:])
        g_b.append(gb)

    # gate-scale Wproj in place; combined projected bias
    for l in range(2):
        nc.vector.tensor_scalar(wproj[l][:], wproj[l][:], g_b[l][:], None,
                                op0=ALU.mult)
    bsum = sb.tile([128, DT], F32)
    btmp = sb.tile([128, DT], F32)
    nc.vector.tensor_scalar(btmp[:], bproj[:, 0, :], g_b[0][:], None, op0=ALU.mult)
    nc.vector.tensor_scalar(bsum[:], bproj[:, 1, :], g_b[1][:], None, op0=ALU.mult)
    nc.vector.tensor_tensor(bsum[:], bsum[:], btmp[:], ALU.add)

    # ---------------- MoE experts + combine + reduce-scatter -------------
    rs_in = dram.tile([S, 512], F32)   # [4 blocks x 512 D-rows, 512 tok]
    rs_out = dram.tile([512, 512], F32)
    for q in range(4):
        t0 = q * 512
        ya = ph.tile([128, 1024], F32, name=f"ya{q}", tag="ph")   # dchunk 0,1
        yb = ph.tile([128, 1024], F32, name=f"yb{q}", tag="ph")   # dchunk 2,3
        yps = [ya[:, 0:512], ya[:, 512:1024], yb[:, 0:512], yb[:, 512:1024]]
        for l in range(2):
            for ft in range(FT):
                phh = ps.tile([128, 512], F32, name=f"ph{q}{l}{ft}", tag="ps")
                for kt in range(DT):
                    nc.tensor.matmul(
                        phh[:], wfc[l][kt][:, ft * 128:(ft + 1) * 128],
                        xln[kt][:, t0:t0 + 512],
                        start=(kt == 0), stop=(kt == DT - 1),
                    )
                hsb = sb.tile([128, 512], BF16, name=f"h{q}{l}{ft}",
                              tag="hsb", bufs=3)
                nc.scalar.activation(hsb[:], phh[:], AF.Gelu_apprx_tanh,
                                     bias=bfc[:, l, ft:ft + 1])
                for dc in range(DT):
                    nc.tensor.matmul(
                        yps[dc],
                        wproj[l][:, ft * 512 + dc * 128:ft * 512 + (dc + 1) * 128],
                        hsb[:],
                        start=(l == 0 and ft == 0), stop=False,
                    )
        for dc in range(DT):
            nc.tensor.matmul(yps[dc], id025[:], xln[dc][:, t0:t0 + 512],
                             start=False, stop=True)
            msb = sb.tile([128, 512], F32, name=f"m{q}{dc}", tag="msb", bufs=3)
            nc.vector.tensor_scalar(msb[:], yps[dc], bsum[:, dc:dc + 1], None,
                                    op0=ALU.add)
            nc.sync.dma_start(rs_in[t0 + dc * 128:t0 + (dc + 1) * 128, :], msb[:])
    nc.gpsimd.collective_compute(
        "ReduceScatter", ALU.add, replica_groups=GROUPS4,
        ins=[rs_in.opt()], outs=[rs_out.opt()],
    )

    # ---------------- layernorm-2 ----------------
    mz = []
    for dt_ in range(DT):
        mz_t = sb.tile([128, 512], F32, name=f"mz{dt_}")
        nc.sync.dma_start(mz_t[:], rs_out[dt_ * 128:(dt_ + 1) * 128, :])
        mz.append(mz_t)
    stats2 = sb.tile([128, 8], F32)
    sq2 = sb.tile([128, 512], F32, name="sq2", tag="msb", bufs=3)
    for dt_ in range(DT):
        nc.scalar.activation(sq2[:], mz[dt_][:], AF.Square,
                             accum_out=stats2[:, 4 + dt_:5 + dt_])
        nc.vector.reduce_sum(stats2[:, dt_:dt_ + 1], mz[dt_][:], axis=AX.X)
    pst2 = ps.tile([1, 8], F32, name="pst2", tag="ps")
    nc.tensor.matmul(pst2[:], ones_col_f[:], stats2[:], start=True, stop=True)
    st8b = sb.tile([1, 8], F32)
    nc.vector.tensor_copy(st8b[:], pst2[:])
    pay2 = sb.tile([1, 16], F32)
    nc.vector.memset(pay2[:], 0.0)
    nc.vector.reduce_sum(pay2[:, 0:1], st8b[:, 0:4], axis=AX.X)
    nc.vector.reduce_sum(pay2[:, 1:2], st8b[:, 4:8], axis=AX.X)
    ar2_in = dram.tile([1, 16], F32)
    ar2_out = dram.tile([1, 16], F32)
    nc.sync.dma_start(ar2_in[:], pay2[:])
    nc.gpsimd.collective_compute(
        "AllReduce", ALU.add, replica_groups=GROUP8,
        ins=[ar2_in.opt()], outs=[ar2_out.opt()],
    )
    gst2 = sb.tile([1, 16], F32)
    nc.sync.dma_start(gst2[:], ar2_out[:])
    s2_d, t2_d = ln_scale_shift(gst2, lnv["lnw2"], lnv["lnb2"], "ln2")
    for dt_ in range(DT):
        osb = sb.tile([128, 512], F32, name=f"o{dt_}", tag="osb", bufs=2)
        nc.vector.tensor_scalar(
            osb[:], mz[dt_][:], s2_d[:, dt_:dt_ + 1], t2_d[:, dt_:dt_ + 1],
            op0=ALU.mult, op1=ALU.add)
        nc.sync.dma_start(t["outT"].ap()[dt_ * 128:(dt_ + 1) * 128, :], osb[:])


def build(repeat=1):
    nc = bacc.Bacc("TRN2", target_bir_lowering=False, debug=False,
                   num_devices=N_CORES)
    t = _declare_io(nc)
    with tile.TileContext(nc) as tc:
        with (
            tc.tile_pool(name="sb", bufs=1) as sb,
            tc.tile_pool(name="ph", bufs=3, space="PSUM") as ph,
            tc.tile_pool(name="ps", bufs=2, space="PSUM") as ps,
            tc.tile_pool(name="dram", bufs=1, space="DRAM") as dram,
        ):
            for _ in range(repeat):
                _emit_body(nc, tc, t, sb, ph, ps, dram)
    nc.compile()
    return nc


# ----------------------------------------------------------------------------
# host-side sharding / gathering
# ----------------------------------------------------------------------------

def _prep_core_inputs(inputs, c):
    b, j = divmod(c, 4)
    h0 = 2 * j           # first of this core's 2 heads
    e0 = 2 * j           # first of this core's 2 experts
    x = f32(inputs["x"])[b]            # [S, D]
    W1 = f32(inputs["W1"])
    b1 = f32(inputs["b1"])
    W2 = f32(inputs["W2"])
    b2 = f32(inputs["b2"])
    Wr = f32(inputs["Wr"])
    br = f32(inputs["br"])
    Wfc = f32(inputs["Wfc"])
    bfc_ = f32(inputs["bfc"])
    Wproj = f32(inputs["Wproj"])
    bproj_ = f32(inputs["bproj"])

    xT = np.ascontiguousarray(x.T)                    # [D, S]
    d = {}
    d["xT_bf"] = bf(xT).reshape(DT, 128, S)
    d["xq"] = (0.25 * xT + 0.25 * b2[:, None]).astype(np.float32).reshape(DT, 128, S)
    qs, ks, vs = 64 * h0, D + 64 * h0, 2 * D + 64 * h0
    w1h = np.concatenate(
        [W1[:, qs:qs + 128], W1[:, ks:ks + 128], W1[:, vs:vs + 128]], axis=1)
    d["w1h"] = bf(w1h).reshape(DT, 128, 384)
    d["b1qk"] = np.stack([b1[qs:qs + 128], b1[ks:ks + 128]], axis=1)
    d["bv_row"] = bf(b1[vs:vs + 128]).reshape(1, 128)
    d["w2h"] = bf(W2[64 * h0:64 * h0 + 128, :])
    d["wr_r"] = np.ascontiguousarray(
        Wr.reshape(DT, 128, E).transpose(1, 0, 2)).reshape(128, DT * E)
    d["br_r"] = br.reshape(1, E)
    for nm, vec in (("lnw1", inputs["ln1_w"]), ("lnb1", inputs["ln1_b"]),
                    ("lnw2", inputs["ln2_w"]), ("lnb2", inputs["ln2_b"])):
        d[nm] = np.ascontiguousarray(f32(vec).reshape(DT, 128).T)
    d["wfc_in"] = bf(Wfc[e0:e0 + 2]).reshape(2, DT, 128, F)
    d["bfc_r"] = np.ascontiguousarray(
        bfc_[e0:e0 + 2].reshape(2, FT, 128).transpose(0, 2, 1))
    d["wproj_in"] = np.ascontiguousarray(
        bf(Wproj[e0:e0 + 2]).reshape(2, FT, 128, 512).transpose(0, 2, 1, 3)
    ).reshape(2, 128, FT * 512)
    d["bproj_r"] = np.ascontiguousarray(
        bproj_[e0:e0 + 2].reshape(2, DT, 128).transpose(0, 2, 1))
    esel = np.zeros((2, 1, E), np.float32)
    esel[0, 0, e0] = 1.0
    esel[1, 0, e0 + 1] = 1.0
    d["esel"] = esel
    # harness passes contiguous float32/bf16 arrays
    d = {k: np.ascontiguousarray(v) for k, v in d.items()}
    return d


def make_in_maps(inputs):
    return [_prep_core_inputs(inputs, c) for c in range(N_CORES)]


def assemble(results):
    out = np.empty((B, S, D), np.float32)
    for c in range(N_CORES):
        b, j = divmod(c, 4)
        out[b, j * 512:(j + 1) * 512, :] = results[c]["outT"].T
    return out


_NC_CACHE = {}


def kernel(**inputs):
    if "nc" not in _NC_CACHE:
        _NC_CACHE["nc"] = build()
    nc = _NC_CACHE["nc"]
    in_maps = make_in_maps(inputs)
    res = run_bass_kernel_spmd(nc, in_maps, core_ids=list(range(N_CORES)))
    return assemble(res.results)


if __name__ == "__main__":
    nc = build()
    print("built ok")


# revision 22
# speedup vs baseline: 1.1064x; 1.1064x over previous
"""Trainium2 Bass kernel for nn_Block_71820443124300 (moe_routing).

Reference computation (B=2, S=2048, D=512, H=8, E=8, F=4D=2048):
  attention (no mask) -> +x -> global layernorm -> dense MoE over all
  experts with softmax(router-mean) gates -> +x -> global layernorm.

Sharding over 8 cores: two groups of 4 cores, one per batch element.
Within a group of 4 cores:
  - attention is head-sharded (2 of 8 heads per core), partial attention
    output all-reduced (with x/4 + b2/4 folded in => z = attn + x).
  - global LN stats via tiny 8-rank all-reduce.
  - MoE is expert-sharded (2 of 8 experts per core); gate-weighted
    partials (+ x_ln1/4 residual via scaled-identity matmul) are
    reduce-scattered so core j ends up owning token block j.
  - final LN stats via tiny 8-rank all-reduce; each core outputs its
    [512 token x 512 feature] block (transposed); host reassembles.

All matmuls run in bf16 (fp32 PSUM accumulation); layernorm statistics,
softmax denominators and residual paths are fp32 except where noted.
Everything is computed in feature-on-partition ("transposed") layout so
no on-device transposes are needed anywhere.
"""

import numpy as np
import ml_dtypes

import concourse.bass as bass
import concourse.bacc as bacc
import concourse.mybir as mybir
import concourse.tile as tile
from concourse.bass_utils import run_bass_kernel_spmd

F32 = mybir.dt.float32
BF16 = mybir.dt.bfloat16
AF = mybir.ActivationFunctionType
ALU = mybir.AluOpType
AX = mybir.AxisListType

B, S, D, H, E = 2, 2048, 512, 8, 8
F = 4 * D            # 2048
DH = D // H          # 64
EPS = 1e-12
N_CORES = 8
DT = D // 128        # 4 D-tiles
ST = S // 128        # 16 token tiles
FT = F // 128        # 16 f tiles
NTOT = float(B * S * D)   # layernorm element count

GROUPS4 = [[0, 1, 2, 3], [4, 5, 6, 7]]
GROUP8 = [list(range(8))]


def bf(x):
    return np.asarray(x, dtype=ml_dtypes.bfloat16)


def f32(x):
    return np.ascontiguousarray(np.asarray(x, dtype=np.float32))


# ----------------------------------------------------------------------------
# device program
# ----------------------------------------------------------------------------

def _declare_io(nc):
    t = {}
    def inp(name, shape, dt):
        t[name] = nc.dram_tensor(name, list(shape), dt, kind="ExternalInput")
    inp("xT_bf", [DT, 128, S], BF16)       # x[b].T as D-tiles
    inp("xq", [DT, 128, S], F32)           # 0.25*x[b].T + 0.25*b2 (per D row)
    inp("w1h", [DT, 128, 384], BF16)       # [q2h | k2h | v2h] columns of W1
    inp("b1qk", [128, 2], F32)             # q,k bias per dh-partition
    inp("bv_row", [1, 128], BF16)          # v bias row
    inp("w2h", [128, 512], BF16)           # W2 rows for this core's 2 heads
    inp("wr_r", [128, 32], F32)            # Wr tiles: col block t = Wr[128t:,:]
    inp("br_r", [1, 8], F32)
    inp("lnw1", [128, DT], F32)
    inp("lnb1", [128, DT], F32)
    inp("lnw2", [128, 1], F32)     # host-sliced D-strip (c%4) of ln2_w
    inp("lnb2", [128, 1], F32)
    inp("wfc_in", [2, DT, 128, F], BF16)   # per local expert, D-tiled
    inp("bfc_r", [2, 128, FT], F32)        # per-f bias, f-tiled
    inp("wproj_in", [2, 128, FT * 512], BF16)  # f-tile-major [128, 8192]
    inp("bproj_r", [2, 128, DT], F32)
    inp("esel", [2, 1, 8], F32)            # one-hot rows selecting local experts
    t["outT"] = nc.dram_tensor("outT", [128, S], F32, kind="ExternalOutput")
    return t


def _emit_body(nc, tc, t, sb, ph, ps, dram, collectives=True):
    def collective(kind, groups, cin, cout):
        if collectives:
            nc.gpsimd.collective_compute(
                kind, ALU.add, replica_groups=groups,
                ins=[cin.opt()], outs=[cout.opt()],
            )
        else:
            n = cout.shape[0]
            nc.sync.dma_start(cout[:], cin[0:n])
    """Emit one full forward pass. sb: SBUF pool, ph: psum pool with
    [128,1024] slots (bufs=3), ps: psum pool with [128,512] slots (bufs=2),
    dram: DRAM pool for collective bounce buffers."""

    # ---------------- constants & small tiles ----------------
    ones_col_f = sb.tile([128, 1], F32)
    nc.vector.memset(ones_col_f[:], 1.0)
    ones_row_f = sb.tile([1, 128], F32)
    nc.vector.memset(ones_row_f[:], 1.0)
    ones_row_b = sb.tile([1, 128], BF16)
    nc.vector.memset(ones_row_b[:], 1.0)
    id025 = sb.tile([128, 128], BF16)
    nc.gpsimd.memset(id025[:], 0.0)
    nc.gpsimd.affine_select(
        out=id025[:], in_=id025[:], compare_op=ALU.not_equal, fill=0.25,
        base=0, pattern=[[-1, 128]], channel_multiplier=1,
    )

    b1qk = sb.tile([128, 2], F32)
    nc.sync.dma_start(b1qk[:], t["b1qk"].ap())
    bv_row = sb.tile([1, 128], BF16)
    nc.sync.dma_start(bv_row[:], t["bv_row"].ap())
    w2h = sb.tile([128, 512], BF16)
    nc.sync.dma_start(w2h[:], t["w2h"].ap())
    wr_r = sb.tile([128, 32], F32)
    nc.sync.dma_start(wr_r[:], t["wr_r"].ap())
    br_r = sb.tile([1, 8], F32)
    nc.sync.dma_start(br_r[:], t["br_r"].ap())
    lnv = {}
    for nm, w_ in (("lnw1", DT), ("lnb1", DT), ("lnw2", 1), ("lnb2", 1)):
        lnv[nm] = sb.tile([128, w_], F32, name=nm + "_sb")
        nc.sync.dma_start(lnv[nm][:], t[nm].ap())
    esel = sb.tile([1, 2, 8], F32)
    for l in range(2):
        nc.sync.dma_start(esel[:, l, :], t["esel"].ap()[l])

    # ---------------- big weight/activation loads ----------------
    # spread the latency-critical first loads across DMA queues
    dma_engines = [nc.sync, nc.scalar, nc.gpsimd, nc.sync]
    xT = []
    for dt_ in range(DT):
        x_t = sb.tile([128, S], BF16, name=f"xT{dt_}")
        dma_engines[dt_].dma_start(x_t[:], t["xT_bf"].ap()[dt_])
        xT.append(x_t)
    w1h = []
    for dt_ in range(DT):
        w_t = sb.tile([128, 384], BF16, name=f"w1h{dt_}")
        dma_engines[dt_].dma_start(w_t[:], t["w1h"].ap()[dt_])
        w1h.append(w_t)

    # ---------------- qkv ----------------
    # qT/kT: [128 (2 heads x 64 dh), S] = W1 slice^T @ x^T
    qT = sb.tile([128, S], BF16)
    kT = sb.tile([128, S], BF16)
    for (dst, col0, bcol) in ((qT, 0, 0), (kT, 128, 1)):
        for half in range(2):
            pqk = ph.tile([128, 1024], F32, name=f"pqk{col0}_{half}", tag="ph")
            for kt in range(DT):
                for qc in range(2):
                    nc.tensor.matmul(
                        pqk[:, qc * 512:(qc + 1) * 512],
                        w1h[kt][:, col0:col0 + 128],
                        xT[kt][:, half * 1024 + qc * 512:half * 1024 + (qc + 1) * 512],
                        start=(kt == 0), stop=(kt == DT - 1),
                    )
            nc.vector.tensor_scalar(
                dst[:, half * 1024:(half + 1) * 1024], pqk[:],
                b1qk[:, bcol:bcol + 1], None, op0=ALU.add)

    # v (un-transposed): [tok, 128 dv] per token tile, then packed into
    # v_aug [128, ST*130]: per tile [h0 64v | 1 | h1 64v | 1]
    v_aug = sb.tile([128, ST * 130], BF16)
    ones_sl = v_aug[:].rearrange("p (n c) -> p n c", c=65)[:, :, 64:65]
    nc.vector.memset(ones_sl, 1.0)
    for st_ in range(ST):
        pv = ps.tile([128, 128], F32, name=f"pv{st_}", tag="ps")
        for kt in range(DT):
            nc.tensor.matmul(
                pv[:], xT[kt][:, st_ * 128:(st_ + 1) * 128],
                w1h[kt][:, 256:384],
                start=(kt == 0), stop=False,
            )
        nc.tensor.matmul(pv[:], ones_row_b[:], bv_row[:], start=False, stop=True)
        dst = v_aug[:, st_ * 130:st_ * 130 + 130].rearrange(
            "p (h c) -> p h c", c=65)[:, :, 0:64]
        nc.vector.tensor_copy(dst, pv[:].rearrange("p (h c) -> p h c", c=64))

    # ---------------- attention: scores -> softmax -> AV ----------------
    oT = sb.tile([128, S], BF16)   # [2*64 feat, q] normalized attention out
    for h in range(2):
        hp = h * 64
        for qh in range(2):
            q0 = qh * 1024
            po = ph.tile([128, 1024], F32, name=f"po{h}_{qh}", tag="ph")
            for kt in range(ST):
                psc = ph.tile([128, 1024], F32, name=f"psc{h}_{qh}_{kt}", tag="ph")
                for qc in range(2):
                    nc.tensor.matmul(
                        psc[:, qc * 512:(qc + 1) * 512],
                        kT[hp:hp + 64, kt * 128:(kt + 1) * 128],
                        qT[hp:hp + 64, q0 + qc * 512:q0 + (qc + 1) * 512],
                        start=True, stop=True,
                    )
                wexp = sb.tile([128, 1024], BF16, name=f"wexp{h}{qh}{kt}",
                               tag="wexp", bufs=2)
                nc.scalar.activation(wexp[:], psc[:], AF.Exp, scale=0.125)
                for qc in range(2):
                    nc.tensor.matmul(
                        po[0:65, qc * 512:(qc + 1) * 512],
                        v_aug[:, kt * 130 + h * 65: kt * 130 + h * 65 + 65],
                        wexp[:, qc * 512:(qc + 1) * 512],
                        start=(kt == 0), stop=(kt == ST - 1),
                    )
            # normalize: rows 0..63 divided by row 64
            recip = sb.tile([1, 1024], F32, name=f"rc{h}{qh}", tag="recip", bufs=2)
            nc.vector.reciprocal(recip[:], po[64:65, :])
            pbc = ph.tile([128, 1024], F32, name=f"pbc{h}_{qh}", tag="ph")
            for qc in range(2):
                nc.tensor.matmul(
                    pbc[0:64, qc * 512:(qc + 1) * 512],
                    ones_row_f[:, 0:64],
                    recip[:, qc * 512:(qc + 1) * 512],
                    start=True, stop=True,
                )
            bc_sb = sb.tile([64, 1024], F32, name=f"bcsb{h}{qh}", tag="bcsb",
                            bufs=1)
            nc.vector.tensor_copy(bc_sb[:], pbc[0:64, :])
            nc.vector.tensor_tensor(
                oT[hp:hp + 64, q0:q0 + 1024], po[0:64, :], bc_sb[:], ALU.mult)

    # deferred bulk loads (needed from the z/MoE phase onward)
    xq = []
    for dt_ in range(DT):
        xq_t = sb.tile([128, S], F32, name=f"xq{dt_}", tag=f"zx{dt_}")
        nc.sync.dma_start(xq_t[:], t["xq"].ap()[dt_])
        xq.append(xq_t)
    wfc = [[None] * DT for _ in range(2)]
    for l in range(2):
        for dt_ in range(DT):
            w_t = sb.tile([128, F], BF16, name=f"wfc{l}_{dt_}")
            nc.sync.dma_start(w_t[:], t["wfc_in"].ap()[l, dt_])
            wfc[l][dt_] = w_t
    wproj = []
    for l in range(2):
        w_t = sb.tile([128, FT * 512], BF16, name=f"wproj{l}")
        nc.sync.dma_start(w_t[:], t["wproj_in"].ap()[l])
        wproj.append(w_t)
    bfc = sb.tile([128, 2, FT], F32)
    bproj = sb.tile([128, 2, DT], F32)
    for l in range(2):
        nc.sync.dma_start(bfc[:, l, :], t["bfc_r"].ap()[l])
        nc.sync.dma_start(bproj[:, l, :], t["bproj_r"].ap()[l])

    # ---------------- attention out-proj + residual; z all-reduce --------
    ar_in = dram.tile([D, S], BF16)
    ar_out = dram.tile([D, S], BF16)
    for mt in range(DT):
        for half in range(2):
            pa = ph.tile([128, 1024], F32, name=f"pa{mt}_{half}", tag="ph")
            for qc in range(2):
                nc.tensor.matmul(
                    pa[:, qc * 512:(qc + 1) * 512],
                    w2h[:, mt * 128:(mt + 1) * 128],
                    oT[:, half * 1024 + qc * 512:half * 1024 + (qc + 1) * 512],
                    start=True, stop=True,
                )
            # += 0.25*x^T + 0.25*b2 (so the 4-rank all-reduce yields z = a+x)
            arsb = sb.tile([128, 1024], BF16, name=f"arsb{mt}_{half}",
                           tag="arsb", bufs=3)
            xqs = xq[mt][:, half * 1024:(half + 1) * 1024]
            nc.vector.tensor_tensor(arsb[:], pa[:], xqs, ALU.add)
            nc.sync.dma_start(
                ar_in[mt * 128:(mt + 1) * 128, half * 1024:(half + 1) * 1024],
                arsb[:])
    collective("AllReduce", GROUPS4, ar_in, ar_out)
    zT = []
    for dt_ in range(DT):
        z_t = sb.tile([128, S], BF16, name=f"zT{dt_}", tag=f"zx{dt_}")
        nc.sync.dma_start(z_t[:], ar_out[dt_ * 128:(dt_ + 1) * 128, :])
        zT.append(z_t)

    # ---------------- layernorm-1 statistics (global over both b) --------
    # cols 0..7: per-(dt,half) row sums; cols 8..15: row sums of squares
    stats_in = sb.tile([128, 16], F32)
    for dt_ in range(DT):
        for hf in range(2):
            zsl = zT[dt_][:, hf * 1024:(hf + 1) * 1024]
            scr = sb.tile([128, 1024], BF16, name=f"sqs{dt_}{hf}", tag="arsb",
                          bufs=3)
            c = 2 * dt_ + hf
            nc.scalar.activation(scr[:], zsl, AF.Square,
                                 accum_out=stats_in[:, 8 + c:9 + c])
            nc.vector.reduce_sum(stats_in[:, c:c + 1], zsl, axis=AX.X)
    pst = ps.tile([1, 16], F32, name="pst", tag="ps")
    nc.tensor.matmul(pst[:], ones_col_f[:], stats_in[:], start=True, stop=True)
    st8 = sb.tile([1, 16], F32)
    nc.vector.tensor_copy(st8[:], pst[:])
    pay = sb.tile([1, 16], F32)
    nc.vector.memset(pay[:], 0.0)
    s1l = sb.tile([1, 1], F32)
    s2l = sb.tile([1, 1], F32)
    nc.vector.reduce_sum(s1l[:], st8[:, 0:8], axis=AX.X)
    nc.vector.reduce_sum(s2l[:], st8[:, 8:16], axis=AX.X)
    nc.vector.tensor_scalar(pay[:, 0:1], s1l[:], 0.25, None, op0=ALU.mult)
    nc.vector.tensor_scalar(pay[:, 1:2], s2l[:], 0.25, None, op0=ALU.mult)
    ar1b_in = dram.tile([1, 16], F32)
    ar1b_out = dram.tile([1, 16], F32)
    nc.sync.dma_start(ar1b_in[:], pay[:])
    collective("AllReduce", GROUP8, ar1b_in, ar1b_out)
    gst = sb.tile([1, 16], F32)
    nc.sync.dma_start(gst[:], ar1b_out[:])

    def ln_scale_shift(gstats, lnw, lnb, tag):
        """gstats [1,16] with [0]=sum,[1]=sumsq -> per-partition scale/shift
        tiles s_d, t_d [128, DT], plus (mu_b, rstd_b) [128,1] broadcasts."""
        mu = sb.tile([1, 1], F32, name=f"mu_{tag}")
        nc.vector.tensor_scalar(mu[:], gstats[:, 0:1], 1.0 / NTOT, None,
                                op0=ALU.mult)
        ex2 = sb.tile([1, 1], F32, name=f"ex2_{tag}")
        nc.vector.tensor_scalar(ex2[:], gstats[:, 1:2], 1.0 / NTOT, None,
                                op0=ALU.mult)
        musq = sb.tile([1, 1], F32, name=f"musq_{tag}")
        nc.vector.tensor_tensor(musq[:], mu[:], mu[:], ALU.mult)
        var = sb.tile([1, 1], F32, name=f"var_{tag}")
        nc.vector.tensor_tensor(var[:], ex2[:], musq[:], ALU.subtract)
        # unbiased (ddof=1): * N/(N-1), then +eps
        nc.vector.tensor_scalar(var[:], var[:], NTOT / (NTOT - 1.0), EPS,
                                op0=ALU.mult, op1=ALU.add)
        std = sb.tile([1, 1], F32, name=f"std_{tag}")
        nc.scalar.activation(std[:], var[:], AF.Sqrt)
        rstd = sb.tile([1, 1], F32, name=f"rstd_{tag}")
        nc.vector.reciprocal(rstd[:], std[:])
        two = sb.tile([1, 2], F32, name=f"two_{tag}")
        nc.vector.tensor_copy(two[:, 0:1], mu[:])
        nc.vector.tensor_copy(two[:, 1:2], rstd[:])
        pb = ps.tile([128, 2], F32, name=f"pb_{tag}", tag="ps")
        nc.tensor.matmul(pb[:], ones_row_f[:], two[:], start=True, stop=True)
        mr_b = sb.tile([128, 2], F32, name=f"mrb_{tag}")
        nc.vector.tensor_copy(mr_b[:], pb[:])
        w_ = lnw.shape[-1]
        s_d = sb.tile([128, w_], F32, name=f"sd_{tag}")
        nc.vector.tensor_scalar(s_d[:], lnw[:], mr_b[:, 1:2], None, op0=ALU.mult)
        t_d = sb.tile([128, w_], F32, name=f"td_{tag}")
        nc.vector.tensor_scalar(t_d[:], s_d[:], mr_b[:, 0:1], None, op0=ALU.mult)
        nc.vector.tensor_tensor(t_d[:], lnb[:], t_d[:], ALU.subtract)
        return s_d, t_d

    s1_d, t1_d = ln_scale_shift(gst, lnv["lnw1"], lnv["lnb1"], "ln1")

    xln = []
    for dt_ in range(DT):
        xl_t = sb.tile([128, S], BF16, name=f"xln{dt_}")
        xln.append(xl_t)
    for qc in range(4):
        for dt_ in range(DT):
            cs = slice(qc * 512, (qc + 1) * 512)
            nc.vector.tensor_scalar(
                xln[dt_][:, cs], zT[dt_][:, cs], s1_d[:, dt_:dt_ + 1],
                t1_d[:, dt_:dt_ + 1], op0=ALU.mult, op1=ALU.add)

    # ---------------- router ----------------
    zrm = sb.tile([128, DT], F32)
    halves = stats_in[:, 0:8].rearrange("p (d two) -> p d two", two=2)
    nc.vector.tensor_tensor(zrm[:], halves[:, :, 0], halves[:, :, 1], ALU.add)
    nc.vector.tensor_scalar(zrm[:], zrm[:], 1.0 / S, None, op0=ALU.mult)
    m1 = sb.tile([128, DT], F32)
    nc.vector.tensor_tensor(m1[:], zrm[:], s1_d[:], ALU.mult)
    nc.vector.tensor_tensor(m1[:], m1[:], t1_d[:], ALU.add)
    plg = ps.tile([1, 8], F32, name="plg", tag="ps")
    for dt_ in range(DT):
        nc.tensor.matmul(plg[:], m1[:, dt_:dt_ + 1], wr_r[:, dt_ * 8:(dt_ + 1) * 8],
                         start=(dt_ == 0), stop=(dt_ == DT - 1))
    lg = sb.tile([1, 8], F32)
    nc.vector.tensor_copy(lg[:], plg[:])
    nc.vector.tensor_tensor(lg[:], lg[:], br_r[:], ALU.add)
    gex = sb.tile([1, 8], F32)
    gden = sb.tile([1, 1], F32)
    nc.scalar.activation(gex[:], lg[:], AF.Exp, accum_out=gden[:])
    grec = sb.tile([1, 1], F32)
    nc.vector.reciprocal(grec[:], gden[:])
    gates = sb.tile([1, 8], F32)
    nc.vector.tensor_scalar(gates[:], gex[:], grec[:], None, op0=ALU.mult)
    g_b = []
    for l in range(2):
        gtmp = sb.tile([1, 8], F32, name=f"gtmp{l}")
        nc.vector.tensor_tensor(gtmp[:], gates[:], esel[:, l, :], ALU.mult)
        gl = sb.tile([1, 1], F32, name=f"gl{l}")
        nc.vector.reduce_sum(gl[:], gtmp[:], axis=AX.X)
        pgb = ps.tile([128, 1], F32, name=f"pgb{l}", tag="ps")
        nc.tensor.matmul(pgb[:], ones_row_f[:], gl[:], start=True, stop=True)
        gb = sb.tile([128, 1], F32, name=f"gb{l}")
        nc.vector.tensor_copy(gb[:], pgb[:])
        g_b.append(gb)

    # gate-scale Wproj in place; combined projected bias
    for l in range(2):
        nc.vector.tensor_scalar(wproj[l][:], wproj[l][:], g_b[l][:], None,
                                op0=ALU.mult)
    bsum = sb.tile([128, DT], F32)
    btmp = sb.tile([128, DT], F32)
    nc.vector.tensor_scalar(btmp[:], bproj[:, 0, :], g_b[0][:], None, op0=ALU.mult)
    nc.vector.tensor_scalar(bsum[:], bproj[:, 1, :], g_b[1][:], None, op0=ALU.mult)
    nc.vector.tensor_tensor(bsum[:], bsum[:], btmp[:], ALU.add)

    # ---------------- MoE experts + combine + reduce-scatter -------------
    # One RS per 512-token block, issued as soon as that block's partials
    # are done, so comm overlaps the next block's compute. RS chunking is
    # along D, so each core ends up owning a 128-row D-strip (index c%4,
    # matching its host-sliced ln2 weights) for all 2048 tokens of its b.
    rs_in = [dram.tile([512, 512], BF16, name=f"rsin{q}") for q in range(4)]
    rs_out = [dram.tile([128, 512], BF16, name=f"rsout{q}") for q in range(4)]
    for q in range(4):
        t0 = q * 512
        ya = ph.tile([128, 1024], F32, name=f"ya{q}", tag="ph")   # dchunk 0,1
        yb = ph.tile([128, 1024], F32, name=f"yb{q}", tag="ph")   # dchunk 2,3
        yps = [ya[:, 0:512], ya[:, 512:1024], yb[:, 0:512], yb[:, 512:1024]]
        for l in range(2):
            for ft in range(FT):
                phh = ps.tile([128, 512], F32, name=f"ph{q}{l}{ft}", tag="ps")
                for kt in range(DT):
                    nc.tensor.matmul(
                        phh[:], wfc[l][kt][:, ft * 128:(ft + 1) * 128],
                        xln[kt][:, t0:t0 + 512],
                        start=(kt == 0), stop=(kt == DT - 1),
                    )
                hsb = sb.tile([128, 512], BF16, name=f"h{q}{l}{ft}",
                              tag="hsb", bufs=3)
                nc.scalar.activation(hsb[:], phh[:], AF.Gelu_apprx_tanh,
                                     bias=bfc[:, l, ft:ft + 1])
                for dc in range(DT):
                    nc.tensor.matmul(
                        yps[dc],
                        wproj[l][:, ft * 512 + dc * 128:ft * 512 + (dc + 1) * 128],
                        hsb[:],
                        start=(l == 0 and ft == 0), stop=False,
                    )
        for dc in range(DT):
            nc.tensor.matmul(yps[dc], id025[:], xln[dc][:, t0:t0 + 512],
                             start=False, stop=True)
            msb = sb.tile([128, 512], BF16, name=f"m{q}{dc}", tag="msb", bufs=3)
            nc.vector.tensor_scalar(msb[:], yps[dc], bsum[:, dc:dc + 1], None,
                                    op0=ALU.add)
            nc.sync.dma_start(rs_in[q][dc * 128:(dc + 1) * 128, :], msb[:])
        collective("ReduceScatter", GROUPS4, rs_in[q], rs_out[q])

    # ---------------- layernorm-2 ----------------
    mz = []
    stats2 = sb.tile([128, 8], F32)
    for q in range(4):
        mz_t = sb.tile([128, 512], BF16, name=f"mz{q}")
        nc.sync.dma_start(mz_t[:], rs_out[q][:])
        mz.append(mz_t)
        sq2 = sb.tile([128, 512], BF16, name=f"sq2_{q}", tag="msb", bufs=3)
        nc.scalar.activation(sq2[:], mz_t[:], AF.Square,
                             accum_out=stats2[:, 4 + q:5 + q])
        nc.vector.reduce_sum(stats2[:, q:q + 1], mz_t[:], axis=AX.X)
    pst2 = ps.tile([1, 8], F32, name="pst2", tag="ps")
    nc.tensor.matmul(pst2[:], ones_col_f[:], stats2[:], start=True, stop=True)
    st8b = sb.tile([1, 8], F32)
    nc.vector.tensor_copy(st8b[:], pst2[:])
    pay2 = sb.tile([1, 16], F32)
    nc.vector.memset(pay2[:], 0.0)
    nc.vector.reduce_sum(pay2[:, 0:1], st8b[:, 0:4], axis=AX.X)
    nc.vector.reduce_sum(pay2[:, 1:2], st8b[:, 4:8], axis=AX.X)
    ar2_in = dram.tile([1, 16], F32)
    ar2_out = dram.tile([1, 16], F32)
    nc.sync.dma_start(ar2_in[:], pay2[:])
    collective("AllReduce", GROUP8, ar2_in, ar2_out)
    gst2 = sb.tile([1, 16], F32)
    nc.sync.dma_start(gst2[:], ar2_out[:])
    s2_d, t2_d = ln_scale_shift(gst2, lnv["lnw2"], lnv["lnb2"], "ln2")
    for q in range(4):
        osb = sb.tile([128, 512], F32, name=f"o{q}", tag="osb", bufs=2)
        nc.vector.tensor_scalar(
            osb[:], mz[q][:], s2_d[:, 0:1], t2_d[:, 0:1],
            op0=ALU.mult, op1=ALU.add)
        nc.sync.dma_start(t["outT"].ap()[:, q * 512:(q + 1) * 512], osb[:])


def build(repeat=1, collectives=True):
    nc = bacc.Bacc("TRN2", target_bir_lowering=False, debug=False,
                   num_devices=N_CORES if collectives else 1)
    t = _declare_io(nc)
    with tile.TileContext(nc) as tc:
        with (
            tc.tile_pool(name="sb", bufs=1) as sb,
            tc.tile_pool(name="ph", bufs=3, space="PSUM") as ph,
            tc.tile_pool(name="ps", bufs=2, space="PSUM") as ps,
            tc.tile_pool(name="dram", bufs=1, space="DRAM") as dram,
        ):
            for _ in range(repeat):
                _emit_body(nc, tc, t, sb, ph, ps, dram, collectives=collectives)
    nc.compile()
    return nc


# ----------------------------------------------------------------------------
# host-side sharding / gathering
# ----------------------------------------------------------------------------

def _prep_core_inputs(inputs, c):
    b, j = divmod(c, 4)
    h0 = 2 * j           # first of this core's 2 heads
    e0 = 2 * j           # first of this core's 2 experts
    x = f32(inputs["x"])[b]            # [S, D]
    W1 = f32(inputs["W1"])
    b1 = f32(inputs["b1"])
    W2 = f32(inputs["W2"])
    b2 = f32(inputs["b2"])
    Wr = f32(inputs["Wr"])
    br = f32(inputs["br"])
    Wfc = f32(inputs["Wfc"])
    bfc_ = f32(inputs["bfc"])
    Wproj = f32(inputs["Wproj"])
    bproj_ = f32(inputs["bproj"])

    xT = np.ascontiguousarray(x.T)                    # [D, S]
    d = {}
    d["xT_bf"] = bf(xT).reshape(DT, 128, S)
    d["xq"] = (0.25 * xT + 0.25 * b2[:, None]).astype(np.float32).reshape(DT, 128, S)
    qs, ks, vs = 64 * h0, D + 64 * h0, 2 * D + 64 * h0
    w1h = np.concatenate(
        [W1[:, qs:qs + 128], W1[:, ks:ks + 128], W1[:, vs:vs + 128]], axis=1)
    d["w1h"] = bf(w1h).reshape(DT, 128, 384)
    d["b1qk"] = np.stack([b1[qs:qs + 128], b1[ks:ks + 128]], axis=1)
    d["bv_row"] = bf(b1[vs:vs + 128]).reshape(1, 128)
    d["w2h"] = bf(W2[64 * h0:64 * h0 + 128, :])
    d["wr_r"] = np.ascontiguousarray(
        Wr.reshape(DT, 128, E).transpose(1, 0, 2)).reshape(128, DT * E)
    d["br_r"] = br.reshape(1, E)
    for nm, vec in (("lnw1", inputs["ln1_w"]), ("lnb1", inputs["ln1_b"])):
        d[nm] = np.ascontiguousarray(f32(vec).reshape(DT, 128).T)
    d["lnw2"] = f32(inputs["ln2_w"])[128 * j:128 * (j + 1)].reshape(128, 1)
    d["lnb2"] = f32(inputs["ln2_b"])[128 * j:128 * (j + 1)].reshape(128, 1)
    d["wfc_in"] = bf(Wfc[e0:e0 + 2]).reshape(2, DT, 128, F)
    d["bfc_r"] = np.ascontiguousarray(
        bfc_[e0:e0 + 2].reshape(2, FT, 128).transpose(0, 2, 1))
    d["wproj_in"] = np.ascontiguousarray(
        bf(Wproj[e0:e0 + 2]).reshape(2, FT, 128, 512).transpose(0, 2, 1, 3)
    ).reshape(2, 128, FT * 512)
    d["bproj_r"] = np.ascontiguousarray(
        bproj_[e0:e0 + 2].reshape(2, DT, 128).transpose(0, 2, 1))
    esel = np.zeros((2, 1, E), np.float32)
    esel[0, 0, e0] = 1.0
    esel[1, 0, e0 + 1] = 1.0
    d["esel"] = esel
    # harness passes contiguous float32/bf16 arrays
    d = {k: np.ascontiguousarray(v) for k, v in d.items()}
    return d


def make_in_maps(inputs):
    return [_prep_core_inputs(inputs, c) for c in range(N_CORES)]


def assemble(results):
    out = np.empty((B, S, D), np.float32)
    for c in range(N_CORES):
        b, j = divmod(c, 4)
        out[b, :, 128 * j:128 * (j + 1)] = results[c]["outT"].T
    return out


_NC_CACHE = {}


def kernel(**inputs):
    if "nc" not in _NC_CACHE:
        _NC_CACHE["nc"] = build()
    nc = _NC_CACHE["nc"]
    in_maps = make_in_maps(inputs)
    res = run_bass_kernel_spmd(nc, in_maps, core_ids=list(range(N_CORES)))
    return assemble(res.results)


if __name__ == "__main__":
    nc = build()
    print("built ok")


# revision 27
# speedup vs baseline: 2.5644x; 2.3179x over previous
"""Trainium2 Bass kernel for nn_Block_71820443124300 (moe_routing).

Reference computation (B=2, S=2048, D=512, H=8, E=8, F=4D=2048):
  attention (no mask) -> +x -> global layernorm -> dense MoE over all
  experts with softmax(router-mean) gates -> +x -> global layernorm.

Sharding over 8 cores: two groups of 4 cores, one per batch element.
Within a group of 4 cores:
  - attention is head-sharded (2 of 8 heads per core), partial attention
    output all-reduced (with x/4 + b2/4 folded in => z = attn + x).
  - global LN stats via tiny 8-rank all-reduce.
  - MoE is expert-sharded (2 of 8 experts per core); gate-weighted
    partials (+ x_ln1/4 residual via scaled-identity matmul) are
    reduce-scattered so core j ends up owning token block j.
  - final LN stats via tiny 8-rank all-reduce; each core outputs its
    [512 token x 512 feature] block (transposed); host reassembles.

All matmuls run in bf16 (fp32 PSUM accumulation); layernorm statistics,
softmax denominators and residual paths are fp32 except where noted.
Everything is computed in feature-on-partition ("transposed") layout so
no on-device transposes are needed anywhere.
"""

import numpy as np
import ml_dtypes

import concourse.bass as bass
import concourse.bacc as bacc
import concourse.mybir as mybir
import concourse.tile as tile
from concourse.bass_utils import run_bass_kernel_spmd

F32 = mybir.dt.float32
BF16 = mybir.dt.bfloat16
AF = mybir.ActivationFunctionType
ALU = mybir.AluOpType
AX = mybir.AxisListType

B, S, D, H, E = 2, 2048, 512, 8, 8
F = 4 * D            # 2048
DH = D // H          # 64
EPS = 1e-12
N_CORES = 8
DT = D // 128        # 4 D-tiles
ST = S // 128        # 16 token tiles
FT = F // 128        # 16 f tiles
NTOT = float(B * S * D)   # layernorm element count

GROUPS4 = [[0, 1, 2, 3], [4, 5, 6, 7]]
GROUP8 = [list(range(8))]


def bf(x):
    return np.asarray(x, dtype=ml_dtypes.bfloat16)


def f32(x):
    return np.ascontiguousarray(np.asarray(x, dtype=np.float32))


# ----------------------------------------------------------------------------
# device program
# ----------------------------------------------------------------------------

def _declare_io(nc):
    t = {}
    def inp(name, shape, dt):
        t[name] = nc.dram_tensor(name, list(shape), dt, kind="ExternalInput")
    inp("xT_bf", [DT, 128, S], BF16)       # x[b].T as D-tiles
    inp("xq", [DT, 128, S], BF16)          # 0.25*x[b].T + 0.25*b2 (per D row)
    inp("w1h", [DT, 128, 384], BF16)       # [q2h | k2h | v2h] columns of W1
    inp("b1qk", [128, 2], F32)             # q,k bias per dh-partition
    inp("bv_row", [1, 128], BF16)          # v bias row
    inp("w2h", [128, 512], BF16)           # W2 rows for this core's 2 heads
    inp("wr_r", [128, 32], F32)            # Wr tiles: col block t = Wr[128t:,:]
    inp("br_r", [1, 8], F32)
    inp("lnw1", [128, DT], F32)
    inp("lnb1", [128, DT], F32)
    inp("lnw2", [128, 1], F32)     # host-sliced D-strip (c%4) of ln2_w
    inp("lnb2", [128, 1], F32)
    inp("wfc_in", [2, DT, 128, F], BF16)   # per local expert, D-tiled, *ln1_w
    inp("cw_r", [2, 128, FT], F32)         # colsum(Wfc*ln1_w), f-tiled
    inp("cbb_r", [2, 128, FT], F32)        # bfc + Wfc^T ln1_b, f-tiled
    inp("wproj_in", [2, 128, FT * 512], BF16)  # f-tile-major [128, 8192]
    inp("bproj_r", [2, 128, DT], F32)
    inp("esel", [2, 1, 8], F32)            # one-hot rows selecting local experts
    t["outT"] = nc.dram_tensor("outT", [128, S], F32, kind="ExternalOutput")
    return t


def _emit_body(nc, tc, t, sb, ph, ps, dram, collectives=True):
    def collective(kind, groups, cin, cout):
        if collectives:
            nc.gpsimd.collective_compute(
                kind, ALU.add, replica_groups=groups,
                ins=[cin.opt()], outs=[cout.opt()],
            )
        else:
            n = cout.shape[0]
            nc.sync.dma_start(cout[:], cin[0:n])
    """Emit one full forward pass. sb: SBUF pool, ph: psum pool with
    [128,1024] slots (bufs=3), ps: psum pool with [128,512] slots (bufs=2),
    dram: DRAM pool for collective bounce buffers."""

    # ---------------- constants & small tiles ----------------
    ones_col_f = sb.tile([128, 1], F32)
    nc.vector.memset(ones_col_f[:], 1.0)
    ones_row_f = sb.tile([1, 128], F32)
    nc.vector.memset(ones_row_f[:], 1.0)
    ones_row_b = sb.tile([1, 128], BF16)
    nc.vector.memset(ones_row_b[:], 1.0)
    id025 = sb.tile([128, 128], BF16)
    nc.gpsimd.memset(id025[:], 0.0)
    nc.gpsimd.affine_select(
        out=id025[:], in_=id025[:], compare_op=ALU.not_equal, fill=0.25,
        base=0, pattern=[[-1, 128]], channel_multiplier=1,
    )

    b1qk = sb.tile([128, 2], F32)
    nc.sync.dma_start(b1qk[:], t["b1qk"].ap())
    bv_row = sb.tile([1, 128], BF16)
    nc.sync.dma_start(bv_row[:], t["bv_row"].ap())
    w2h = sb.tile([128, 512], BF16)
    nc.sync.dma_start(w2h[:], t["w2h"].ap())
    wr_r = sb.tile([128, 32], F32)
    nc.sync.dma_start(wr_r[:], t["wr_r"].ap())
    br_r = sb.tile([1, 8], F32)
    nc.sync.dma_start(br_r[:], t["br_r"].ap())
    lnv = {}
    for nm, w_ in (("lnw1", DT), ("lnb1", DT), ("lnw2", 1), ("lnb2", 1)):
        lnv[nm] = sb.tile([128, w_], F32, name=nm + "_sb")
        nc.sync.dma_start(lnv[nm][:], t[nm].ap())
    esel = sb.tile([1, 2, 8], F32)
    for l in range(2):
        nc.sync.dma_start(esel[:, l, :], t["esel"].ap()[l])

    # ---------------- big weight/activation loads ----------------
    # spread the latency-critical first loads across DMA queues
    dma_engines = [nc.sync, nc.scalar, nc.gpsimd, nc.sync]
    xT = []
    for dt_ in range(DT):
        x_t = sb.tile([128, S], BF16, name=f"xT{dt_}")
        dma_engines[dt_].dma_start(x_t[:], t["xT_bf"].ap()[dt_])
        xT.append(x_t)
    w1h = []
    for dt_ in range(DT):
        w_t = sb.tile([128, 384], BF16, name=f"w1h{dt_}")
        dma_engines[dt_].dma_start(w_t[:], t["w1h"].ap()[dt_])
        w1h.append(w_t)

    # ---------------- qkv ----------------
    # qT/kT: [128 (2 heads x 64 dh), S] = W1 slice^T @ x^T
    qT = sb.tile([128, S], BF16)
    kT = sb.tile([128, S], BF16)
    for (dst, col0, bcol) in ((qT, 0, 0), (kT, 128, 1)):
        for half in range(2):
            pqk = ph.tile([128, 1024], F32, name=f"pqk{col0}_{half}", tag="ph")
            for kt in range(DT):
                for qc in range(2):
                    nc.tensor.matmul(
                        pqk[:, qc * 512:(qc + 1) * 512],
                        w1h[kt][:, col0:col0 + 128],
                        xT[kt][:, half * 1024 + qc * 512:half * 1024 + (qc + 1) * 512],
                        start=(kt == 0), stop=(kt == DT - 1),
                    )
            nc.vector.tensor_scalar(
                dst[:, half * 1024:(half + 1) * 1024], pqk[:],
                b1qk[:, bcol:bcol + 1], None, op0=ALU.add)

    # v (un-transposed): [tok, 128 dv] per token tile, then packed into
    # v_aug [128, ST*130]: per tile [h0 64v | 1 | h1 64v | 1]
    v_aug = sb.tile([128, ST * 130], BF16)
    ones_sl = v_aug[:].rearrange("p (n c) -> p n c", c=65)[:, :, 64:65]
    nc.vector.memset(ones_sl, 1.0)
    for st_ in range(ST):
        pv = ps.tile([128, 128], F32, name=f"pv{st_}", tag="ps")
        for kt in range(DT):
            nc.tensor.matmul(
                pv[:], xT[kt][:, st_ * 128:(st_ + 1) * 128],
                w1h[kt][:, 256:384],
                start=(kt == 0), stop=False,
            )
        nc.tensor.matmul(pv[:], ones_row_b[:], bv_row[:], start=False, stop=True)
        dst = v_aug[:, st_ * 130:st_ * 130 + 130].rearrange(
            "p (h c) -> p h c", c=65)[:, :, 0:64]
        nc.vector.tensor_copy(dst, pv[:].rearrange("p (h c) -> p h c", c=64))

    # deferred bulk loads (needed from the out-proj / MoE phases onward)
    xq = []
    for dt_ in range(DT):
        xq_t = sb.tile([128, S], BF16, name=f"xq{dt_}")
        nc.sync.dma_start(xq_t[:], t["xq"].ap()[dt_])
        xq.append(xq_t)
    wfc = [[None] * DT for _ in range(2)]
    for l in range(2):
        for dt_ in range(DT):
            w_t = sb.tile([128, F], BF16, name=f"wfc{l}_{dt_}")
            nc.sync.dma_start(w_t[:], t["wfc_in"].ap()[l, dt_])
            wfc[l][dt_] = w_t
    wproj = []
    for l in range(2):
        w_t = sb.tile([128, FT * 512], BF16, name=f"wproj{l}")
        nc.sync.dma_start(w_t[:], t["wproj_in"].ap()[l])
        wproj.append(w_t)
    cw_sb = sb.tile([128, 2, FT], F32)
    cbb_sb = sb.tile([128, 2, FT], F32)
    bproj = sb.tile([128, 2, DT], F32)
    for l in range(2):
        nc.sync.dma_start(cw_sb[:, l, :], t["cw_r"].ap()[l])
        nc.sync.dma_start(cbb_sb[:, l, :], t["cbb_r"].ap()[l])
        nc.sync.dma_start(bproj[:, l, :], t["bproj_r"].ap()[l])

    # ---- attention (q-halves outer) + per-half z all-reduce ----
    # After each 1024-query half is finished (both heads), its out-proj +
    # residual partial is pushed into a 4-rank bf16 all-reduce, so the first
    # half's collective overlaps the second half's attention compute.
    oT = sb.tile([128, S], BF16)   # [2*64 feat, q] normalized attention out
    ar_in = [dram.tile([D, 1024], BF16, name=f"arin{i}") for i in range(2)]
    ar_out = [dram.tile([D, 1024], BF16, name=f"arout{i}") for i in range(2)]
    zT = []
    for dt_ in range(DT):
        z_t = sb.tile([128, S], BF16, name=f"zT{dt_}")
        zT.append(z_t)
    # layernorm-1 statistics: cols 0..7 per-(dt,half) row sums; 8..15 sumsq
    stats_in = sb.tile([128, 16], F32)

    for qh in range(2):
        q0 = qh * 1024
        for h in range(2):
            hp = h * 64
            po = ph.tile([128, 1024], F32, name=f"po{h}_{qh}", tag="ph")
            for kt in range(ST):
                psc = ph.tile([128, 1024], F32, name=f"psc{h}_{qh}_{kt}", tag="ph")
                for qc in range(2):
                    nc.tensor.matmul(
                        psc[:, qc * 512:(qc + 1) * 512],
                        kT[hp:hp + 64, kt * 128:(kt + 1) * 128],
                        qT[hp:hp + 64, q0 + qc * 512:q0 + (qc + 1) * 512],
                        start=True, stop=True,
                    )
                wexp = sb.tile([128, 1024], BF16, name=f"wexp{h}{qh}{kt}",
                               tag="wexp", bufs=2)
                nc.scalar.activation(wexp[:], psc[:], AF.Exp, scale=0.125)
                for qc in range(2):
                    nc.tensor.matmul(
                        po[0:65, qc * 512:(qc + 1) * 512],
                        v_aug[:, kt * 130 + h * 65: kt * 130 + h * 65 + 65],
                        wexp[:, qc * 512:(qc + 1) * 512],
                        start=(kt == 0), stop=(kt == ST - 1),
                    )
            # normalize: rows 0..63 divided by row 64
            recip = sb.tile([1, 1024], F32, name=f"rc{h}{qh}", tag="recip", bufs=2)
            nc.vector.reciprocal(recip[:], po[64:65, :])
            pbc = ph.tile([128, 1024], F32, name=f"pbc{h}_{qh}", tag="ph")
            for qc in range(2):
                nc.tensor.matmul(
                    pbc[0:64, qc * 512:(qc + 1) * 512],
                    ones_row_f[:, 0:64],
                    recip[:, qc * 512:(qc + 1) * 512],
                    start=True, stop=True,
                )
            bc_sb = sb.tile([64, 1024], F32, name=f"bcsb{h}{qh}", tag="bcsb",
                            bufs=1)
            nc.vector.tensor_copy(bc_sb[:], pbc[0:64, :])
            nc.vector.tensor_tensor(
                oT[hp:hp + 64, q0:q0 + 1024], po[0:64, :], bc_sb[:], ALU.mult)

        # out-proj + 0.25*(x + b2) residual for this half, then all-reduce
        for mt in range(DT):
            pa = ph.tile([128, 1024], F32, name=f"pa{mt}_{qh}", tag="ph")
            for qc in range(2):
                nc.tensor.matmul(
                    pa[:, qc * 512:(qc + 1) * 512],
                    w2h[:, mt * 128:(mt + 1) * 128],
                    oT[:, q0 + qc * 512:q0 + (qc + 1) * 512],
                    start=True, stop=True,
                )
            arsb = sb.tile([128, 1024], BF16, name=f"arsb{mt}_{qh}",
                           tag="arsb", bufs=3)
            nc.vector.tensor_tensor(arsb[:], pa[:],
                                    xq[mt][:, q0:q0 + 1024], ALU.add)
            nc.sync.dma_start(ar_in[qh][mt * 128:(mt + 1) * 128, :], arsb[:])
        collective("AllReduce", GROUPS4, ar_in[qh], ar_out[qh])
        for dt_ in range(DT):
            nc.sync.dma_start(zT[dt_][:, q0:q0 + 1024],
                              ar_out[qh][dt_ * 128:(dt_ + 1) * 128, :])
        for dt_ in range(DT):
            zsl = zT[dt_][:, q0:q0 + 1024]
            scr = sb.tile([128, 1024], BF16, name=f"sqs{dt_}{qh}", tag="arsb",
                          bufs=3)
            c = 2 * dt_ + qh
            nc.scalar.activation(scr[:], zsl, AF.Square,
                                 accum_out=stats_in[:, 8 + c:9 + c])
            nc.vector.reduce_sum(stats_in[:, c:c + 1], zsl, axis=AX.X)

    pst = ps.tile([1, 16], F32, name="pst", tag="ps")
    nc.tensor.matmul(pst[:], ones_col_f[:], stats_in[:], start=True, stop=True)
    st8 = sb.tile([1, 16], F32)
    nc.vector.tensor_copy(st8[:], pst[:])
    pay = sb.tile([1, 16], F32)
    nc.vector.memset(pay[:], 0.0)
    s1l = sb.tile([1, 1], F32)
    s2l = sb.tile([1, 1], F32)
    nc.vector.reduce_sum(s1l[:], st8[:, 0:8], axis=AX.X)
    nc.vector.reduce_sum(s2l[:], st8[:, 8:16], axis=AX.X)
    nc.vector.tensor_scalar(pay[:, 0:1], s1l[:], 0.25, None, op0=ALU.mult)
    nc.vector.tensor_scalar(pay[:, 1:2], s2l[:], 0.25, None, op0=ALU.mult)
    ar1b_in = dram.tile([1, 16], F32)
    ar1b_out = dram.tile([1, 16], F32)
    nc.sync.dma_start(ar1b_in[:], pay[:])
    collective("AllReduce", GROUP8, ar1b_in, ar1b_out)
    gst = sb.tile([1, 16], F32)
    nc.sync.dma_start(gst[:], ar1b_out[:])

    def ln_scale_shift(gstats, lnw, lnb, tag):
        """gstats [1,16] with [0]=sum,[1]=sumsq -> per-partition scale/shift
        s_d, t_d (width of lnw), plus mrb [128,3] = bcast(mu, rstd, mu*rstd)."""
        mu = sb.tile([1, 1], F32, name=f"mu_{tag}")
        nc.vector.tensor_scalar(mu[:], gstats[:, 0:1], 1.0 / NTOT, None,
                                op0=ALU.mult)
        ex2 = sb.tile([1, 1], F32, name=f"ex2_{tag}")
        nc.vector.tensor_scalar(ex2[:], gstats[:, 1:2], 1.0 / NTOT, None,
                                op0=ALU.mult)
        musq = sb.tile([1, 1], F32, name=f"musq_{tag}")
        nc.vector.tensor_tensor(musq[:], mu[:], mu[:], ALU.mult)
        var = sb.tile([1, 1], F32, name=f"var_{tag}")
        nc.vector.tensor_tensor(var[:], ex2[:], musq[:], ALU.subtract)
        # unbiased (ddof=1): * N/(N-1), then +eps
        nc.vector.tensor_scalar(var[:], var[:], NTOT / (NTOT - 1.0), EPS,
                                op0=ALU.mult, op1=ALU.add)
        std = sb.tile([1, 1], F32, name=f"std_{tag}")
        nc.scalar.activation(std[:], var[:], AF.Sqrt)
        rstd = sb.tile([1, 1], F32, name=f"rstd_{tag}")
        nc.vector.reciprocal(rstd[:], std[:])
        three = sb.tile([1, 3], F32, name=f"three_{tag}")
        nc.vector.tensor_copy(three[:, 0:1], mu[:])
        nc.vector.tensor_copy(three[:, 1:2], rstd[:])
        nc.vector.tensor_tensor(three[:, 2:3], mu[:], rstd[:], ALU.mult)
        pb = ps.tile([128, 3], F32, name=f"pb_{tag}", tag="ps")
        nc.tensor.matmul(pb[:], ones_row_f[:], three[:], start=True, stop=True)
        mrb = sb.tile([128, 3], F32, name=f"mrb_{tag}")
        nc.vector.tensor_copy(mrb[:], pb[:])
        w_ = lnw.shape[-1]
        s_d = sb.tile([128, w_], F32, name=f"sd_{tag}")
        nc.vector.tensor_scalar(s_d[:], lnw[:], mrb[:, 1:2], None, op0=ALU.mult)
        t_d = sb.tile([128, w_], F32, name=f"td_{tag}")
        nc.vector.tensor_scalar(t_d[:], s_d[:], mrb[:, 0:1], None, op0=ALU.mult)
        nc.vector.tensor_tensor(t_d[:], lnb[:], t_d[:], ALU.subtract)
        return s_d, t_d, mrb

    s1_d, t1_d, mrb1 = ln_scale_shift(gst, lnv["lnw1"], lnv["lnb1"], "ln1")

    # gelu input correction: gelu(rstd*h_pre + cf), cf = (bfc + Wfc^T b_ln)
    # - mu*rstd*colsum(Wfc*w_ln)  [h_pre = (Wfc*w_ln)^T z runs stats-free]
    cf_all = sb.tile([128, 2, FT], F32)
    nc.vector.tensor_scalar(cf_all[:], cw_sb[:], mrb1[:, 2:3], None,
                            op0=ALU.mult)
    nc.vector.tensor_tensor(cf_all[:], cbb_sb[:], cf_all[:], ALU.subtract)

    # residual matmul weights: diag(0.25 * s1_d[:, dc]) per D-chunk
    id_s1 = []
    for dc in range(DT):
        idt = sb.tile([128, 128], BF16, name=f"ids1_{dc}")
        nc.vector.tensor_scalar(idt[:], id025[:], s1_d[:, dc:dc + 1], None,
                                op0=ALU.mult)
        id_s1.append(idt)

    # ---------------- router ----------------
    zrm = sb.tile([128, DT], F32)
    halves = stats_in[:, 0:8].rearrange("p (d two) -> p d two", two=2)
    nc.vector.tensor_tensor(zrm[:], halves[:, :, 0], halves[:, :, 1], ALU.add)
    nc.vector.tensor_scalar(zrm[:], zrm[:], 1.0 / S, None, op0=ALU.mult)
    m1 = sb.tile([128, DT], F32)
    nc.vector.tensor_tensor(m1[:], zrm[:], s1_d[:], ALU.mult)
    nc.vector.tensor_tensor(m1[:], m1[:], t1_d[:], ALU.add)
    plg = ps.tile([1, 8], F32, name="plg", tag="ps")
    for dt_ in range(DT):
        nc.tensor.matmul(plg[:], m1[:, dt_:dt_ + 1], wr_r[:, dt_ * 8:(dt_ + 1) * 8],
                         start=(dt_ == 0), stop=(dt_ == DT - 1))
    lg = sb.tile([1, 8], F32)
    nc.vector.tensor_copy(lg[:], plg[:])
    nc.vector.tensor_tensor(lg[:], lg[:], br_r[:], ALU.add)
    gex = sb.tile([1, 8], F32)
    gden = sb.tile([1, 1], F32)
    nc.scalar.activation(gex[:], lg[:], AF.Exp, accum_out=gden[:])
    grec = sb.tile([1, 1], F32)
    nc.vector.reciprocal(grec[:], gden[:])
    gates = sb.tile([1, 8], F32)
    nc.vector.tensor_scalar(gates[:], gex[:], grec[:], None, op0=ALU.mult)
    g_b = []
    for l in range(2):
        gtmp = sb.tile([1, 8], F32, name=f"gtmp{l}")
        nc.vector.tensor_tensor(gtmp[:], gates[:], esel[:, l, :], ALU.mult)
        gl = sb.tile([1, 1], F32, name=f"gl{l}")
        nc.vector.reduce_sum(gl[:], gtmp[:], axis=AX.X)
        pgb = ps.tile([128, 1], F32, name=f"pgb{l}", tag="ps")
        nc.tensor.matmul(pgb[:], ones_row_f[:], gl[:], start=True, stop=True)
        gb = sb.tile([128, 1], F32, name=f"gb{l}")
        nc.vector.tensor_copy(gb[:], pgb[:])
        g_b.append(gb)

    # gate-scale Wproj in place; combined bias (gated bproj + 0.25*t1_d)
    for l in range(2):
        nc.vector.tensor_scalar(wproj[l][:], wproj[l][:], g_b[l][:], None,
                                op0=ALU.mult)
    bsum = sb.tile([128, DT], F32)
    btmp = sb.tile([128, DT], F32)
    nc.vector.tensor_scalar(btmp[:], bproj[:, 0, :], g_b[0][:], None, op0=ALU.mult)
    nc.vector.tensor_scalar(bsum[:], bproj[:, 1, :], g_b[1][:], None, op0=ALU.mult)
    nc.vector.tensor_tensor(bsum[:], bsum[:], btmp[:], ALU.add)
    nc.vector.tensor_scalar(btmp[:], t1_d[:], 0.25, None, op0=ALU.mult)
    nc.vector.tensor_tensor(bsum[:], bsum[:], btmp[:], ALU.add)

    # ---------------- MoE experts + combine + reduce-scatter -------------
    # h_pre matmuls depend only on z (stats deferred into the gelu's
    # scale/bias), so the PE streams MoE work while the tiny stats
    # all-reduce is in flight. The first token block's h_pre tiles are
    # staged through SBUF to decouple PE from the stats-gated gelu.
    # One RS per 512-token block, issued as soon as that block's partials
    # are done, so comm overlaps the next block's compute. RS chunking is
    # along D, so each core ends up owning a 128-row D-strip (index c%4,
    # matching its host-sliced ln2 weights) for all 2048 tokens of its b.
    rs_in = [dram.tile([512, 512], BF16, name=f"rsin{q}") for q in range(4)]
    rs_out = [dram.tile([128, 512], BF16, name=f"rsout{q}") for q in range(4)]
    for q in range(4):
        t0 = q * 512
        ya = ph.tile([128, 1024], F32, name=f"ya{q}", tag="ph")   # dchunk 0,1
        yb = ph.tile([128, 1024], F32, name=f"yb{q}", tag="ph")   # dchunk 2,3
        yps = [ya[:, 0:512], ya[:, 512:1024], yb[:, 0:512], yb[:, 512:1024]]
        for l in range(2):
            for ft in range(FT):
                phh = ps.tile([128, 512], F32, name=f"ph{q}{l}{ft}", tag="ps")
                for kt in range(DT):
                    nc.tensor.matmul(
                        phh[:], wfc[l][kt][:, ft * 128:(ft + 1) * 128],
                        zT[kt][:, t0:t0 + 512],
                        start=(kt == 0), stop=(kt == DT - 1),
                    )
                hsb = sb.tile([128, 512], BF16, name=f"h{q}{l}{ft}",
                              tag="hsb", bufs=4)
                if q == 0:
                    hpre = sb.tile([128, 512], BF16, name=f"hp{l}{ft}",
                                   tag="hpre", bufs=12)
                    nc.scalar.copy(hpre[:], phh[:])
                    nc.scalar.activation(hsb[:], hpre[:], AF.Gelu_apprx_tanh,
                                         bias=cf_all[:, l, ft:ft + 1],
                                         scale=mrb1[:, 1:2])
                else:
                    nc.scalar.activation(hsb[:], phh[:], AF.Gelu_apprx_tanh,
                                         bias=cf_all[:, l, ft:ft + 1],
                                         scale=mrb1[:, 1:2])
                for dc in range(DT):
                    nc.tensor.matmul(
                        yps[dc],
                        wproj[l][:, ft * 512 + dc * 128:ft * 512 + (dc + 1) * 128],
                        hsb[:],
                        start=(l == 0 and ft == 0), stop=False,
                    )
        for dc in range(DT):
            nc.tensor.matmul(yps[dc], id_s1[dc][:], zT[dc][:, t0:t0 + 512],
                             start=False, stop=True)
            msb = sb.tile([128, 512], BF16, name=f"m{q}{dc}", tag="msb", bufs=3)
            nc.vector.tensor_scalar(msb[:], yps[dc], bsum[:, dc:dc + 1], None,
                                    op0=ALU.add)
            nc.sync.dma_start(rs_in[q][dc * 128:(dc + 1) * 128, :], msb[:])
        collective("ReduceScatter", GROUPS4, rs_in[q], rs_out[q])

    # ---------------- layernorm-2 ----------------
    mz = []
    stats2 = sb.tile([128, 8], F32)
    for q in range(4):
        mz_t = sb.tile([128, 512], BF16, name=f"mz{q}")
        nc.sync.dma_start(mz_t[:], rs_out[q][:])
        mz.append(mz_t)
        sq2 = sb.tile([128, 512], BF16, name=f"sq2_{q}", tag="msb", bufs=3)
        nc.scalar.activation(sq2[:], mz_t[:], AF.Square,
                             accum_out=stats2[:, 4 + q:5 + q])
        nc.vector.reduce_sum(stats2[:, q:q + 1], mz_t[:], axis=AX.X)
    pst2 = ps.tile([1, 8], F32, name="pst2", tag="ps")
    nc.tensor.matmul(pst2[:], ones_col_f[:], stats2[:], start=True, stop=True)
    st8b = sb.tile([1, 8], F32)
    nc.vector.tensor_copy(st8b[:], pst2[:])
    pay2 = sb.tile([1, 16], F32)
    nc.vector.memset(pay2[:], 0.0)
    nc.vector.reduce_sum(pay2[:, 0:1], st8b[:, 0:4], axis=AX.X)
    nc.vector.reduce_sum(pay2[:, 1:2], st8b[:, 4:8], axis=AX.X)
    ar2_in = dram.tile([1, 16], F32)
    ar2_out = dram.tile([1, 16], F32)
    nc.sync.dma_start(ar2_in[:], pay2[:])
    collective("AllReduce", GROUP8, ar2_in, ar2_out)
    gst2 = sb.tile([1, 16], F32)
    nc.sync.dma_start(gst2[:], ar2_out[:])
    s2_d, t2_d, _mrb2 = ln_scale_shift(gst2, lnv["lnw2"], lnv["lnb2"], "ln2")
    for q in range(4):
        osb = sb.tile([128, 512], F32, name=f"o{q}", tag="osb", bufs=2)
        nc.vector.tensor_scalar(
            osb[:], mz[q][:], s2_d[:, 0:1], t2_d[:, 0:1],
            op0=ALU.mult, op1=ALU.add)
        nc.sync.dma_start(t["outT"].ap()[:, q * 512:(q + 1) * 512], osb[:])


def build(repeat=1, collectives=True):
    nc = bacc.Bacc("TRN2", target_bir_lowering=False, debug=False,
                   num_devices=N_CORES if collectives else 1)
    t = _declare_io(nc)
    with tile.TileContext(nc) as tc:
        with (
            tc.tile_pool(name="sb", bufs=1) as sb,
            tc.tile_pool(name="ph", bufs=3, space="PSUM") as ph,
            tc.tile_pool(name="ps", bufs=2, space="PSUM") as ps,
            tc.tile_pool(name="dram", bufs=1, space="DRAM") as dram,
        ):
            for _ in range(repeat):
                _emit_body(nc, tc, t, sb, ph, ps, dram, collectives=collectives)
    nc.compile()
    return nc


# ----------------------------------------------------------------------------
# host-side sharding / gathering
# ----------------------------------------------------------------------------

def _prep_core_inputs(inputs, c):
    b, j = divmod(c, 4)
    h0 = 2 * j           # first of this core's 2 heads
    e0 = 2 * j           # first of this core's 2 experts
    x = f32(inputs["x"])[b]            # [S, D]
    W1 = f32(inputs["W1"])
    b1 = f32(inputs["b1"])
    W2 = f32(inputs["W2"])
    b2 = f32(inputs["b2"])
    Wr = f32(inputs["Wr"])
    br = f32(inputs["br"])
    Wfc = f32(inputs["Wfc"])
    bfc_ = f32(inputs["bfc"])
    Wproj = f32(inputs["Wproj"])
    bproj_ = f32(inputs["bproj"])

    xT = np.ascontiguousarray(x.T)                    # [D, S]
    d = {}
    d["xT_bf"] = bf(xT).reshape(DT, 128, S)
    d["xq"] = bf(0.25 * xT + 0.25 * b2[:, None]).reshape(DT, 128, S)
    qs, ks, vs = 64 * h0, D + 64 * h0, 2 * D + 64 * h0
    w1h = np.concatenate(
        [W1[:, qs:qs + 128], W1[:, ks:ks + 128], W1[:, vs:vs + 128]], axis=1)
    d["w1h"] = bf(w1h).reshape(DT, 128, 384)
    d["b1qk"] = np.stack([b1[qs:qs + 128], b1[ks:ks + 128]], axis=1)
    d["bv_row"] = bf(b1[vs:vs + 128]).reshape(1, 128)
    d["w2h"] = bf(W2[64 * h0:64 * h0 + 128, :])
    d["wr_r"] = np.ascontiguousarray(
        Wr.reshape(DT, 128, E).transpose(1, 0, 2)).reshape(128, DT * E)
    d["br_r"] = br.reshape(1, E)
    for nm, vec in (("lnw1", inputs["ln1_w"]), ("lnb1", inputs["ln1_b"])):
        d[nm] = np.ascontiguousarray(f32(vec).reshape(DT, 128).T)
    d["lnw2"] = f32(inputs["ln2_w"])[128 * j:128 * (j + 1)].reshape(128, 1)
    d["lnb2"] = f32(inputs["ln2_b"])[128 * j:128 * (j + 1)].reshape(128, 1)
    ln1w = f32(inputs["ln1_w"])
    ln1b = f32(inputs["ln1_b"])
    Wfc_w = Wfc[e0:e0 + 2] * ln1w[None, :, None]
    d["wfc_in"] = bf(Wfc_w).reshape(2, DT, 128, F)
    cw = Wfc_w.sum(axis=1)                                   # [2, F]
    cbb = bfc_[e0:e0 + 2] + np.einsum("d,edf->ef", ln1b, Wfc[e0:e0 + 2])
    d["cw_r"] = np.ascontiguousarray(
        cw.reshape(2, FT, 128).transpose(0, 2, 1)).astype(np.float32)
    d["cbb_r"] = np.ascontiguousarray(
        cbb.reshape(2, FT, 128).transpose(0, 2, 1)).astype(np.float32)
    d["wproj_in"] = np.ascontiguousarray(
        bf(Wproj[e0:e0 + 2]).reshape(2, FT, 128, 512).transpose(0, 2, 1, 3)
    ).reshape(2, 128, FT * 512)
    d["bproj_r"] = np.ascontiguousarray(
        bproj_[e0:e0 + 2].reshape(2, DT, 128).transpose(0, 2, 1))
    esel = np.zeros((2, 1, E), np.float32)
    esel[0, 0, e0] = 1.0
    esel[1, 0, e0 + 1] = 1.0
    d["esel"] = esel
    # harness passes contiguous float32/bf16 arrays
    d = {k: np.ascontiguousarray(v) for k, v in d.items()}
    return d


def make_in_maps(inputs):
    return [_prep_core_inputs(inputs, c) for c in range(N_CORES)]


def assemble(results):
    out = np.empty((B, S, D), np.float32)
    for c in range(N_CORES):
        b, j = divmod(c, 4)
        out[b, :, 128 * j:128 * (j + 1)] = results[c]["outT"].T
    return out


_NC_CACHE = {}


def kernel(**inputs):
    if "nc" not in _NC_CACHE:
        _NC_CACHE["nc"] = build()
    nc = _NC_CACHE["nc"]
    in_maps = make_in_maps(inputs)
    res = run_bass_kernel_spmd(nc, in_maps, core_ids=list(range(N_CORES)))
    return assemble(res.results)


if __name__ == "__main__":
    nc = build()
    print("built ok")
